# revision 27
# baseline (speedup 1.0000x reference)
"""Trainium2 Bass kernel for nn_GAT_30331059044728 (GATv2 message passing).

Self-contained: hardcodes shapes/sharding; only imports concourse from the
container install at /opt/trn_rl_repo.

Strategy (8 NeuronCores):
  * Nodes assigned to (core, window, slot) by a host-side degree-balancing
    permutation: nodes sorted by in-degree, dealt round-robin so every core
    sees the same per-window edge counts (minimizes cross-core tile padding).
    Core d owns 6272 nodes = 49 windows x 128 slots (dst-partitioning);
    edges live on the core owning their dst.
  * Encoder/decoder MLPs data-parallel over nodes, feature-on-partition
    layout, bf16 matmuls.
  * Per GAT layer: each core computes xl=(h@wl+bl) for its block, AllGather
    builds the full xl table [50176,256] bf16 in DRAM; xr=(h@wr+br) stays
    in SBUF.
  * Per-edge xl rows fetched with gpsimd dma_gather (512B rows); int16
    gather indices force a lo/hi half-table phase split at 25088.
  * Per 128-edge tile: host-precomputed onehot mats (prefetched from DRAM in
    16-tile chunks): smT[slot,e] gathers xr[dst] via matmul accumulated on
    top of an identity-matmul pass of xl (s = xl+xr in PSUM); m=LeakyReLU(s)
    on scalar; logits e=sum(att*m) per head on vector; p=exp(e); weighted
    messages p*xl scatter-added into a PSUM window accumulator via the
    second onehot mat sm[e,slot]; softmax denominator accumulated as 3 extra
    matmul columns (softmax without max-subtraction).
  * Window finalize: head-mean of num/den + bias + relu -> h_next; the
    transpose of h_next and the NEXT phase's dense matmuls (xl/xr build of
    layer l+1, or the decoder) are folded per-window into the edge phase.
"""

import sys

sys.path.insert(0, "/opt/trn_rl_repo")

import numpy as np
from concourse import bacc, bass, mybir, tile

F32 = mybir.dt.float32
BF16 = mybir.dt.bfloat16
I16 = mybir.dt.int16

# problem constants
N = 50000
E = 500000
IDIM = 128
HLD = 64
ODIM = 8
H = 3
SLOPE = 0.2
NCORES = 8
NPB = 6272              # nodes per core (49*128)
NPAD = NCORES * NPB     # 50176
W = NPB // 128          # 49 windows
LO = NPAD // 2          # 25088 int16 split
TILE = 128
GCALL = 32              # tiles per dma_gather call (4096 idxs)
FDIM = H * HLD          # 192
ACC_COLS = FDIM + H     # 195
XLP = 256               # padded bf16 xl row (512B)

LAST_EXEC_NS = None


def _bf16(a):
    import ml_dtypes
    return np.ascontiguousarray(np.asarray(a, np.float32)).astype(
        ml_dtypes.bfloat16)


# ----------------------------------------------------------------------------
# host-side edge plan
# ----------------------------------------------------------------------------

def build_edge_plan(edgeIdx):
    src0 = np.ascontiguousarray(edgeIdx[0]).astype(np.int64)
    dst0 = np.ascontiguousarray(edgeIdx[1]).astype(np.int64)

    # degree-balancing node permutation: sort by in-degree, super-groups of
    # 1024 dealt round-robin to cores -> per-(core, window) edge counts are
    # nearly equal across cores.  perm[new_id] = old_id.
    deg = np.bincount(dst0, minlength=NPAD)
    order = np.argsort(-deg, kind="stable")
    grp = order.reshape(W, 128, NCORES)          # [window, slot, core]
    perm = np.ascontiguousarray(
        np.transpose(grp, (2, 0, 1))).reshape(-1)  # [core, window, slot]
    inv = np.empty(NPAD, np.int64)
    inv[perm] = np.arange(NPAD)
    src = inv[src0]
    dst = inv[dst0]

    order_e = np.argsort(dst, kind="stable")
    s_all, d_all = src[order_e], dst[order_e]
    core_ofs = np.searchsorted(d_all, np.arange(NCORES + 1) * NPB)

    # per-core grouped edge lists keyed (phase, window)
    per_core = []
    counts = np.zeros((NCORES, W, 2), np.int64)
    for d in range(NCORES):
        s_d = s_all[core_ofs[d]:core_ofs[d + 1]]
        t_d = d_all[core_ofs[d]:core_ofs[d + 1]] - d * NPB
        w_d = t_d >> 7
        # phase = which half of its owner's block src sits in; table H1 holds
        # every block's first 3136 rows, H2 the second halves.
        ph_d = ((s_d % NPB) >= (NPB // 2)).astype(np.int64)
        key = ph_d * W + w_d
        o2 = np.argsort(key, kind="stable")
        s_d, t_d, key = s_d[o2], t_d[o2], key[o2]
        ofs = np.searchsorted(key, np.arange(2 * W + 1))
        per_core.append((s_d, t_d, ofs))
        cnt = ofs[1:] - ofs[:-1]
        counts[d, :, 0] = cnt[:W]
        counts[d, :, 1] = cnt[W:]

    ntiles = np.maximum(np.ceil(counts.max(axis=0) / TILE).astype(np.int64), 1)
    T_lo = int(ntiles[:, 0].sum())
    T_hi = int(ntiles[:, 1].sum())
    T = T_lo + T_hi
    lo_start = np.concatenate([[0], np.cumsum(ntiles[:, 0])])
    hi_start = np.concatenate([[0], np.cumsum(ntiles[:, 1])]) + T_lo

    idx_lo = np.zeros((NCORES, T_lo * TILE), np.int64)
    idx_hi = np.zeros((NCORES, T_hi * TILE), np.int64)
    dstw = np.full((NCORES, T * TILE), -1.0, np.float32)
    for d in range(NCORES):
        s_d, t_d, ofs = per_core[d]
        for w in range(W):
            for ph in range(2):
                a, b = ofs[ph * W + w], ofs[ph * W + w + 1]
                n = b - a
                stream_t = lo_start[w] if ph == 0 else hi_start[w]
                sb = stream_t * TILE
                blk = s_d[a:b] // NPB
                off = s_d[a:b] % NPB
                hidx = blk * (NPB // 2) + off % (NPB // 2)
                if ph == 0:
                    idx_lo[d, sb:sb + n] = hidx
                else:
                    idx_hi[d, (sb - T_lo * TILE):(sb - T_lo * TILE) + n] = hidx
                dstw[d, sb:sb + n] = (t_d[a:b] & 127).astype(np.float32)

    def wrap16(arr):  # [n] -> [128, n//16]: 16-partition wrap replicated x8
        w = arr.reshape(-1, 16).T.astype(np.int16)
        return np.ascontiguousarray(np.tile(w, (8, 1)))

    return {
        "ntiles": ntiles, "T_lo": T_lo, "T_hi": T_hi, "T": T,
        "lo_start": lo_start, "hi_start": hi_start, "perm": perm,
        "idx_lo_w": [wrap16(idx_lo[d]) for d in range(NCORES)],
        "idx_hi_w": [wrap16(idx_hi[d]) for d in range(NCORES)],
        "dstw": dstw,
    }


def plan_calls(n_tiles):
    calls, t = [], 0
    while t < n_tiles:
        n = min(GCALL, n_tiles - t)
        calls.append((t, n))
        t += n
    return calls


# ----------------------------------------------------------------------------
# device program
# ----------------------------------------------------------------------------

def build_nc(meta, debug=False, nlayers=2):
    ntiles = meta["ntiles"]
    T_lo, T_hi, T = meta["T_lo"], meta["T_hi"], meta["T"]
    lo_start, hi_start = meta["lo_start"], meta["hi_start"]

    nc = bacc.Bacc("TRN2", target_bir_lowering=False, debug=debug,
                   num_swdge_queues=2)

    # ---- I/O ----
    xT = nc.dram_tensor("xT", [IDIM, NPB], BF16, kind="ExternalInput")
    ew = {}
    for item in [
        ("enc_w0", [IDIM, HLD], BF16), ("enc_b0", [HLD, 1]),
        ("enc_w1", [HLD, HLD], BF16), ("enc_b1", [HLD, 1]),
        ("dec_w0", [HLD, HLD], BF16), ("dec_b0", [HLD, 1]),
        ("dec_w1", [HLD, ODIM], BF16), ("dec_b1", [ODIM, 1]),
        ("wl0", [HLD + 1, FDIM], BF16), ("wr0", [HLD + 1, FDIM], BF16),
        ("wl1", [HLD + 1, FDIM], BF16), ("wr1", [HLD + 1, FDIM], BF16),
        ("att0", [128, 4 * FDIM], BF16), ("att1", [128, 4 * FDIM], BF16),
        ("gbias0", [128, HLD]), ("gbias1", [128, HLD]),
        ("identb", [128, 128], BF16),
    ]:
        nm, shape = item[0], item[1]
        dt = item[2] if len(item) > 2 else F32
        ew[nm] = nc.dram_tensor(nm, shape, dt, kind="ExternalInput")
    idx_lo_d = nc.dram_tensor("idx_lo", [128, T_lo * 8], I16, kind="ExternalInput")
    idx_hi_d = nc.dram_tensor("idx_hi", [128, T_hi * 8], I16, kind="ExternalInput")
    smt_d = nc.dram_tensor("smt", [128, T * TILE], BF16, kind="ExternalInput")
    smat_d = nc.dram_tensor("smat", [128, T * TILE], BF16, kind="ExternalInput")
    out_d = nc.dram_tensor("outT", [ODIM, NPB], F32, kind="ExternalOutput")

    # internal DRAM (half-tables H1/H2, double-buffered across layers)
    xl_local = nc.dram_tensor("xl_local", [NPB, XLP], BF16)
    tabs = [[nc.dram_tensor("tab%d_%d" % (h, p), [LO, XLP], BF16,
                            addr_space="Shared")
             for h in range(2)] for p in range(2)]

    lo_calls = plan_calls(T_lo)
    hi_calls = plan_calls(T_hi)

    with tile.TileContext(nc) as tc:
        import contextlib
        stack = contextlib.ExitStack()
        pool = lambda *a, **k: stack.enter_context(tc.tile_pool(*a, **k))
        constp = pool(name="const", bufs=1)
        tabp = pool(name="tables", bufs=1)
        bigp = pool(name="bigbuf", bufs=1)
        xtp = pool(name="xt", bufs=3)
        htp = pool(name="ht", bufs=1)
        hnp = pool(name="hnext", bufs=1)
        stagep = pool(name="stage", bufs=4)
        glop = pool(name="glo", bufs=2)
        ghip = pool(name="ghi", bufs=2)
        smtp = pool(name="smt", bufs=2)
        smatp = pool(name="smat", bufs=2)
        edgep = pool(name="edge", bufs=4)
        decp = pool(name="dec", bufs=2)
        finp = pool(name="fin", bufs=2)
        psp = pool(name="ps", bufs=2, space="PSUM")
        psaccp = pool(name="psacc", bufs=2, space="PSUM")
        pssqp = pool(name="pssq", bufs=3, space="PSUM")
        if True:
            # ---- persistent SBUF ----
            consts = {}
            for nm in ["enc_w0", "enc_b0", "enc_w1", "enc_b1", "dec_w0",
                       "dec_b0", "dec_w1", "dec_b1", "wl0", "wr0", "wl1",
                       "wr1", "att0", "att1", "gbias0", "gbias1", "identb"]:
                t = constp.tile(list(ew[nm].shape), ew[nm].dtype, tag=nm)
                nc.sync.dma_start(t[:], ew[nm][:])
                consts[nm] = t

            ilo_sb = tabp.tile([128, T_lo * 8], I16, tag="ilo")
            nc.sync.dma_start(ilo_sb[:], idx_lo_d[:])
            ihi_sb = tabp.tile([128, T_hi * 8], I16, tag="ihi")
            nc.sync.dma_start(ihi_sb[:], idx_hi_d[:])
            xr_sbs = [tabp.tile([128, W * FDIM], BF16, tag="xr_sb%d" % i,
                                name="xr_sb%d" % i)
                      for i in range(2)]

            # ---- encoder ----
            h0T = bigp.tile([HLD, NPB], BF16, tag="big")
            for t in range(W):
                xt_t = xtp.tile([IDIM, 128], BF16, tag="xt")
                nc.sync.dma_start(xt_t[:], xT[:, t * 128:(t + 1) * 128])
                ps = psp.tile([HLD, 128], F32, tag="ps")
                nc.tensor.matmul(ps[:], lhsT=consts["enc_w0"][:],
                                 rhs=xt_t[:], start=True, stop=True)
                nc.scalar.activation(h0T[:, t * 128:(t + 1) * 128], ps[:],
                                     mybir.ActivationFunctionType.Relu,
                                     bias=consts["enc_b0"][:, 0:1])
            hT = htp.tile([HLD + 1, NPB], BF16, tag="hT")
            nc.vector.memset(hT[HLD:HLD + 1, :], 1.0)
            for t in range(W):
                ps = psp.tile([HLD, 128], F32, tag="ps")
                nc.tensor.matmul(ps[:], lhsT=consts["enc_w1"][:],
                                 rhs=h0T[:, t * 128:(t + 1) * 128],
                                 start=True, stop=True)
                nc.scalar.activation(hT[0:HLD, t * 128:(t + 1) * 128], ps[:],
                                     mybir.ActivationFunctionType.Relu,
                                     bias=consts["enc_b1"][:, 0:1])

            HALF = NPB // 2

            def emit_ag(half, layer):
                nc.gpsimd.collective_compute(
                    "AllGather", mybir.AluOpType.bypass,
                    replica_groups=[list(range(NCORES))],
                    ins=[xl_local[half * HALF:(half + 1) * HALF, :]],
                    outs=[tabs[layer % 2][half][:]],
                )

            def build_lw(hT_cur, t, layer):
                """xl/xr dense build for window t of `layer` from hT_cur.
                Emits the half-table AllGathers as their inputs complete."""
                wl = consts["wl%d" % layer]
                wr = consts["wr%d" % layer]
                lhsT = hT_cur[:, t * 128:(t + 1) * 128]
                psl = psp.tile([128, FDIM], F32, tag="ps")
                nc.tensor.matmul(psl[:], lhsT=lhsT, rhs=wl[:],
                                 start=True, stop=True)
                st = stagep.tile([128, FDIM], BF16, tag="bldst")
                nc.scalar.activation(st[:], psl[:],
                                     mybir.ActivationFunctionType.Copy)
                nc.sync.dma_start(xl_local[t * 128:(t + 1) * 128, 0:FDIM],
                                  st[:])
                psr = psp.tile([128, FDIM], F32, tag="ps")
                nc.tensor.matmul(psr[:], lhsT=lhsT, rhs=wr[:],
                                 start=True, stop=True)
                nc.scalar.activation(
                    xr_sbs[layer % 2][:, t * FDIM:(t + 1) * FDIM], psr[:],
                    mybir.ActivationFunctionType.Copy)
                if t == (W // 2):
                    emit_ag(0, layer)
                elif t == W - 1:
                    emit_ag(1, layer)

            def decode_win(hT_cur, t):
                """decoder MLP for window t, folded into the edge phase."""
                ps0 = psp.tile([HLD, 128], F32, tag="ps")
                nc.tensor.matmul(ps0[:], lhsT=consts["dec_w0"][:],
                                 rhs=hT_cur[0:HLD, t * 128:(t + 1) * 128],
                                 start=True, stop=True)
                y0 = decp.tile([HLD, 128], BF16, tag="y0")
                nc.scalar.activation(y0[:], ps0[:],
                                     mybir.ActivationFunctionType.Relu,
                                     bias=consts["dec_b0"][:, 0:1])
                ps1 = psp.tile([ODIM, 128], F32, tag="ps")
                nc.tensor.matmul(ps1[:], lhsT=consts["dec_w1"][:],
                                 rhs=y0[:], start=True, stop=True)
                yo = decp.tile([ODIM, 128], F32, tag="yo")
                nc.scalar.activation(yo[:], ps1[:],
                                     mybir.ActivationFunctionType.Relu,
                                     bias=consts["dec_b1"][:, 0:1])
                nc.sync.dma_start(out_d[:, t * 128:(t + 1) * 128], yo[:])

            # upfront build of layer 0 (layer 1 builds fold into edge 0)
            for t in range(W):
                build_lw(hT, t, 0)

            # ---- GAT layers ----
            for layer in range(nlayers):
                att = consts["att%d" % layer]
                gbias = consts["gbias%d" % layer]
                xr_sb = xr_sbs[layer % 2]
                tabH1 = tabs[layer % 2][0]
                tabH2 = tabs[layer % 2][1]

                # gather + onehot-mat prefetch (emitted lazily)
                lo_bufs = [None] * len(lo_calls)
                hi_bufs = [None] * len(hi_calls)

                def emit(ci, calls, bufs, idx_sb, tab, pool_, qn, gtag):
                    t0, n = calls[ci]
                    b = pool_.tile([128, GCALL, XLP], BF16, tag=gtag)
                    nc.gpsimd.dma_gather(
                        b[:, 0:n, :], tab, idx_sb[:, t0 * 8:(t0 + n) * 8],
                        n * TILE, n * TILE, XLP, single_packet=False,
                        queue_num=qn)
                    smt = smtp.tile([128, GCALL * TILE], BF16, tag="smt")
                    sm = smatp.tile([128, GCALL * TILE], BF16, tag="smat")
                    if gtag == "ghi":
                        t0 = t0 + T_lo
                    nc.sync.dma_start(
                        smt[:, 0:n * TILE],
                        smt_d[:, t0 * TILE:(t0 + n) * TILE])
                    nc.sync.dma_start(
                        sm[:, 0:n * TILE],
                        smat_d[:, t0 * TILE:(t0 + n) * TILE])
                    bufs[ci] = (b, smt, sm)

                next_lo = [0]
                next_hi = [0]

                def get_chunk(stream_t, nb):
                    """views (xl[128,nb,192], smt[128,nb*128], sm[128,nb*128])
                    for nb consecutive stream tiles starting at stream_t."""
                    if stream_t < T_lo:
                        ci, off = divmod(stream_t, GCALL)
                        while next_lo[0] <= ci:
                            emit(next_lo[0], lo_calls, lo_bufs, ilo_sb,
                                 tabH1[:], glop, 0, "glo")
                            next_lo[0] += 1
                        b, smt, sm = lo_bufs[ci]
                    else:
                        st = stream_t - T_lo
                        ci, off = divmod(st, GCALL)
                        while next_hi[0] <= ci:
                            emit(next_hi[0], hi_calls, hi_bufs, ihi_sb,
                                 tabH2[:], ghip, 1, "ghi")
                            next_hi[0] += 1
                        b, smt, sm = hi_bufs[ci]
                    return (b[:, off:off + nb, 0:FDIM],
                            smt[:, off * TILE:(off + nb) * TILE],
                            sm[:, off * TILE:(off + nb) * TILE])

                h_next = hnp.tile([128, W * HLD], BF16, tag="hnext")
                hTn = htp.tile([HLD + 1, NPB], BF16, tag="hT")
                nc.vector.memset(hTn[HLD:HLD + 1, :], 1.0)

                def window_chunks(w, ph):
                    chunks = []
                    base = lo_start[w] if ph == 0 else hi_start[w]
                    cnt = ntiles[w, ph]
                    t = base
                    while t < base + cnt:
                        pos = t if t < T_lo else t - T_lo
                        nb = min(base + cnt - t, 4, GCALL - pos % GCALL)
                        chunks.append((int(t), int(nb)))
                        t += nb
                    return chunks

                def edge_chunk(b0, nb, xr_win, acc, wi, first, last):
                    xl_q, smt_q, sm_q = get_chunk(b0, nb)
                    wm_q = edgep.tile([128, 4, 200], BF16, tag="wm")
                    m0_q = edgep.tile([128, 4, FDIM], BF16, tag="m0")
                    # s = xl + xr[dst] in PSUM (identity pass + onehot matmul)
                    for sb0 in range(0, nb, 2):
                        ns = min(2, nb - sb0)
                        psq = pssqp.tile([128, 2, FDIM], F32, tag="sqd")
                        nc.tensor.matmul(
                            psq[:, 0:ns, :], lhsT=consts["identb"][:],
                            rhs=xl_q[:, sb0:sb0 + ns, :],
                            start=True, stop=False)
                        for t in range(ns):
                            tt = sb0 + t
                            nc.tensor.matmul(
                                psq[:, t, :],
                                lhsT=smt_q[:, tt * TILE:(tt + 1) * TILE],
                                rhs=xr_win, start=False, stop=True)
                        nc.scalar.activation(
                            m0_q[:, sb0:sb0 + ns, :], psq[:, 0:ns, :],
                            mybir.ActivationFunctionType.Prelu, alpha=SLOPE)
                    m_q = edgep.tile([128, 4, FDIM], BF16, tag="m")
                    nc.vector.tensor_tensor(
                        out=m_q[:, 0:nb, :],
                        in0=m0_q[:, 0:nb, :],
                        in1=att[:].rearrange("p (t f) -> p t f", t=4)
                            [:, 0:nb, :],
                        op=mybir.AluOpType.mult)
                    pe_q = edgep.tile([128, 4, 8], F32, tag="pe")
                    nc.vector.tensor_reduce(
                        out=pe_q[:, 0:nb, 4:7],
                        in_=m_q[:, 0:nb, :].rearrange(
                            "p t (h c) -> p t h c", h=H),
                        axis=mybir.AxisListType.X,
                        op=mybir.AluOpType.add)
                    nc.scalar.activation(
                        wm_q[:, 0:nb, FDIM:FDIM + H], pe_q[:, 0:nb, 4:7],
                        mybir.ActivationFunctionType.Exp)
                    nc.vector.tensor_tensor(
                        out=wm_q[:, 0:nb, 0:FDIM].rearrange(
                            "p t (h c) -> p t h c", h=H),
                        in0=xl_q.rearrange("p t (h c) -> p t h c", h=H),
                        in1=wm_q[:, 0:nb, FDIM:FDIM + H].broadcast_to(
                            [128, nb, H, HLD]),
                        op=mybir.AluOpType.mult)
                    for t in range(nb):
                        nc.tensor.matmul(
                            acc[:, wi, 0:ACC_COLS],
                            lhsT=sm_q[:, t * TILE:(t + 1) * TILE],
                            rhs=wm_q[:, t, 0:ACC_COLS],
                            start=(first and t == 0),
                            stop=(last and t == nb - 1))

                for wp in range(0, W, 2):
                    wpair = [w for w in (wp, wp + 1) if w < W]
                    acc = psaccp.tile([128, 2, 256], F32, tag="acc")
                    for wi, w in enumerate(wpair):
                        xr_win = xr_sb[:, w * FDIM:(w + 1) * FDIM]
                        chunks = (window_chunks(w, 0)
                                  + window_chunks(w, 1))
                        nch = len(chunks)
                        for ci_, (b0, nb) in enumerate(chunks):
                            edge_chunk(b0, nb, xr_win, acc, wi,
                                       ci_ == 0, ci_ == nch - 1)

                    # ---- finalize window pair ----
                    np_ = len(wpair)
                    fin = finp.tile([128, 2, 8], F32, tag="fin")
                    nc.vector.tensor_scalar(
                        out=fin[:, 0:np_, 0:3], in0=acc[:, 0:np_, 192:195],
                        scalar1=3.0, scalar2=1e-16,
                        op0=mybir.AluOpType.mult, op1=mybir.AluOpType.add)
                    nc.vector.reciprocal(fin[:, 0:np_, 4:7],
                                         fin[:, 0:np_, 0:3])
                    u_t = finp.tile([128, 2, FDIM], F32, tag="u")
                    for h in range(H):
                        nc.vector.tensor_tensor(
                            out=u_t[:, 0:np_, h * HLD:(h + 1) * HLD],
                            in0=acc[:, 0:np_, h * HLD:(h + 1) * HLD],
                            in1=fin[:, 0:np_, 4 + h:5 + h].broadcast_to(
                                [128, np_, HLD]),
                            op=mybir.AluOpType.mult)
                    v_t = finp.tile([128, 2, HLD], F32, tag="v")
                    nc.vector.tensor_tensor(
                        out=v_t[:, 0:np_, :], in0=u_t[:, 0:np_, 0:HLD],
                        in1=u_t[:, 0:np_, HLD:2 * HLD],
                        op=mybir.AluOpType.add)
                    v2_t = finp.tile([128, 2, HLD], F32, tag="v2")
                    nc.vector.tensor_tensor(
                        out=v2_t[:, 0:np_, :], in0=v_t[:, 0:np_, :],
                        in1=u_t[:, 0:np_, 2 * HLD:3 * HLD],
                        op=mybir.AluOpType.add)
                    v3_t = finp.tile([128, 2, HLD], F32, tag="v3")
                    nc.vector.tensor_tensor(
                        out=v3_t[:, 0:np_, :], in0=v2_t[:, 0:np_, :],
                        in1=gbias[:].rearrange("p (o f) -> p o f", o=1)
                            .broadcast_to([128, np_, HLD]),
                        op=mybir.AluOpType.add)
                    nc.scalar.activation(
                        h_next[:, wp * HLD:(wp + np_) * HLD],
                        v3_t[:, 0:np_, :],
                        mybir.ActivationFunctionType.Relu)

                    # ---- fold transpose + next dense phase per window ----
                    for w in wpair:
                        pst = psp.tile([HLD, 128], BF16, tag="pst", bufs=1)
                        nc.tensor.transpose(
                            pst[:], in_=h_next[:, w * HLD:(w + 1) * HLD],
                            identity=consts["identb"][:])
                        nc.scalar.activation(
                            hTn[0:HLD, w * 128:(w + 1) * 128], pst[:],
                            mybir.ActivationFunctionType.Copy)
                        if layer + 1 < nlayers:
                            build_lw(hTn, w, layer + 1)
                        else:
                            decode_win(hTn, w)

                hT = hTn

        stack.close()

    nc.compile()
    return nc


# ----------------------------------------------------------------------------
# host orchestration
# ----------------------------------------------------------------------------

def make_in_maps(inputs, plan):
    x = np.asarray(inputs["x"], np.float32)
    xpad = np.zeros((NPAD, IDIM), np.float32)
    xpad[:N] = x
    xpad = xpad[plan["perm"]]

    def col(b):
        return np.ascontiguousarray(np.asarray(b, np.float32).reshape(-1, 1))

    def wplus(wname, bname):
        wm = np.asarray(inputs[wname], np.float32)
        bm = np.asarray(inputs[bname], np.float32)
        return _bf16(np.vstack([wm, bm[None, :]]))

    shared = {
        "enc_w0": _bf16(inputs["enc_w0"]), "enc_b0": col(inputs["enc_b0"]),
        "enc_w1": _bf16(inputs["enc_w1"]), "enc_b1": col(inputs["enc_b1"]),
        "dec_w0": _bf16(inputs["dec_w0"]), "dec_b0": col(inputs["dec_b0"]),
        "dec_w1": _bf16(inputs["dec_w1"]), "dec_b1": col(inputs["dec_b1"]),
        "wl0": wplus("gat0_wl", "gat0_bl"),
        "wr0": wplus("gat0_wr", "gat0_br"),
        "wl1": wplus("gat1_wl", "gat1_bl"),
        "wr1": wplus("gat1_wr", "gat1_br"),
        "att0": _bf16(np.tile(np.asarray(inputs["gat0_att"], np.float32)
                              .reshape(1, FDIM), (128, 4))),
        "att1": _bf16(np.tile(np.asarray(inputs["gat1_att"], np.float32)
                              .reshape(1, FDIM), (128, 4))),
        "gbias0": np.ascontiguousarray(
            np.tile(np.asarray(inputs["gat0_bias"], np.float32)
                    .reshape(1, HLD), (128, 1))),
        "gbias1": np.ascontiguousarray(
            np.tile(np.asarray(inputs["gat1_bias"], np.float32)
                    .reshape(1, HLD), (128, 1))),
        "identb": _bf16(np.eye(128, dtype=np.float32)),
    }
    maps = []
    for d in range(NCORES):
        m = dict(shared)
        m["xT"] = _bf16(xpad[d * NPB:(d + 1) * NPB].T)
        m["idx_lo"] = plan["idx_lo_w"][d]
        m["idx_hi"] = plan["idx_hi_w"][d]
        dw = plan["dstw"][d]  # [T*TILE] float (dst-in-window, -1 pad)
        # smat[p=edge-in-tile, t*128+slot] = 1{dst(edge p of tile t)==slot}
        sm = (dw.reshape(-1, TILE).T[:, :, None]
              == np.arange(128, dtype=np.float32)[None, None])
        m["smat"] = _bf16(sm.reshape(TILE, -1))
        # smt[p=slot, t*128+e] = 1{dst(edge e of tile t)==slot}
        smt = (np.arange(128, dtype=np.float32)[:, None] == dw[None, :])
        m["smt"] = _bf16(smt)
        maps.append(m)
    return maps


def kernel(**inputs):
    global LAST_EXEC_NS
    trace = inputs.pop("trace", False)
    tmpdir = inputs.pop("tmpdir", None)
    from concourse.bass_utils import run_bass_kernel_spmd

    plan = build_edge_plan(np.asarray(inputs["edgeIdx"]))
    nc = build_nc(plan)
    in_maps = make_in_maps(inputs, plan)
    res = run_bass_kernel_spmd(nc, in_maps, list(range(NCORES)),
                               tmpdir=tmpdir, trace=trace)
    LAST_EXEC_NS = res.exec_time_ns
    outs = res.results
    full = np.concatenate([outs[d]["outT"].T for d in range(NCORES)], 0)
    out = np.empty((NPAD, ODIM), np.float32)
    out[plan["perm"]] = full
    return np.ascontiguousarray(out[:N]).astype(np.float32)


# revision 28
# speedup vs baseline: 1.2210x; 1.2210x over previous
"""Trainium2 Bass kernel for nn_GAT_30331059044728 (GATv2 message passing).

Self-contained: hardcodes shapes/sharding; only imports concourse from the
container install at /opt/trn_rl_repo.

Strategy (8 NeuronCores):
  * Nodes assigned to (core, window, slot) by a host-side degree-balancing
    permutation: nodes sorted by in-degree, dealt round-robin so every core
    sees the same per-window edge counts (minimizes cross-core tile padding).
    Core d owns 6272 nodes = 49 windows x 128 slots (dst-partitioning);
    edges live on the core owning their dst.
  * Encoder/decoder MLPs data-parallel over nodes, feature-on-partition
    layout, bf16 matmuls.
  * Per GAT layer: each core computes xl=(h@wl+bl) for its block, AllGather
    builds the full xl table [50176,256] bf16 in DRAM; xr=(h@wr+br) stays
    in SBUF.
  * Per-edge xl rows fetched with gpsimd dma_gather (512B rows); int16
    gather indices force a lo/hi half-table phase split at 25088.
  * Per 128-edge tile: host-precomputed onehot mats (prefetched from DRAM in
    16-tile chunks): smT[slot,e] gathers xr[dst] via matmul accumulated on
    top of an identity-matmul pass of xl (s = xl+xr in PSUM); m=LeakyReLU(s)
    on scalar; logits e=sum(att*m) per head on vector; p=exp(e); weighted
    messages p*xl scatter-added into a PSUM window accumulator via the
    second onehot mat sm[e,slot]; softmax denominator accumulated as 3 extra
    matmul columns (softmax without max-subtraction).
  * Window finalize: head-mean of num/den + bias + relu -> h_next; the
    transpose of h_next and the NEXT phase's dense matmuls (xl/xr build of
    layer l+1, or the decoder) are folded per-window into the edge phase.
"""

import sys

sys.path.insert(0, "/opt/trn_rl_repo")

import numpy as np
from concourse import bacc, bass, mybir, tile

F32 = mybir.dt.float32
BF16 = mybir.dt.bfloat16
I16 = mybir.dt.int16

# problem constants
N = 50000
E = 500000
IDIM = 128
HLD = 64
ODIM = 8
H = 3
SLOPE = 0.2
NCORES = 8
NPB = 6272              # nodes per core (49*128)
NPAD = NCORES * NPB     # 50176
W = NPB // 128          # 49 windows
LO = NPAD // 2          # 25088 int16 split
TILE = 128
GCALL = 16              # tiles per dma_gather call (2048 idxs)
FDIM = H * HLD          # 192
ACC_COLS = FDIM + H     # 195
XLP = 256               # padded bf16 xl row (512B)

LAST_EXEC_NS = None


def _bf16(a):
    import ml_dtypes
    return np.ascontiguousarray(np.asarray(a, np.float32)).astype(
        ml_dtypes.bfloat16)


# ----------------------------------------------------------------------------
# host-side edge plan
# ----------------------------------------------------------------------------

def build_edge_plan(edgeIdx):
    src0 = np.ascontiguousarray(edgeIdx[0]).astype(np.int64)
    dst0 = np.ascontiguousarray(edgeIdx[1]).astype(np.int64)

    # degree-balancing node permutation: sort by in-degree, super-groups of
    # 1024 dealt round-robin to cores -> per-(core, window) edge counts are
    # nearly equal across cores.  perm[new_id] = old_id.
    deg = np.bincount(dst0, minlength=NPAD)
    order = np.argsort(-deg, kind="stable")
    grp = order.reshape(W, 128, NCORES)          # [window, slot, core]
    perm = np.ascontiguousarray(
        np.transpose(grp, (2, 0, 1))).reshape(-1)  # [core, window, slot]
    inv = np.empty(NPAD, np.int64)
    inv[perm] = np.arange(NPAD)
    src = inv[src0]
    dst = inv[dst0]

    order_e = np.argsort(dst, kind="stable")
    s_all, d_all = src[order_e], dst[order_e]
    core_ofs = np.searchsorted(d_all, np.arange(NCORES + 1) * NPB)

    # per-core grouped edge lists keyed (phase, window)
    per_core = []
    counts = np.zeros((NCORES, W, 2), np.int64)
    for d in range(NCORES):
        s_d = s_all[core_ofs[d]:core_ofs[d + 1]]
        t_d = d_all[core_ofs[d]:core_ofs[d + 1]] - d * NPB
        w_d = t_d >> 7
        # phase = which half of its owner's block src sits in; table H1 holds
        # every block's first 3136 rows, H2 the second halves.
        ph_d = ((s_d % NPB) >= (NPB // 2)).astype(np.int64)
        key = ph_d * W + w_d
        o2 = np.argsort(key, kind="stable")
        s_d, t_d, key = s_d[o2], t_d[o2], key[o2]
        ofs = np.searchsorted(key, np.arange(2 * W + 1))
        per_core.append((s_d, t_d, ofs))
        cnt = ofs[1:] - ofs[:-1]
        counts[d, :, 0] = cnt[:W]
        counts[d, :, 1] = cnt[W:]

    ntiles = np.maximum(np.ceil(counts.max(axis=0) / TILE).astype(np.int64), 1)
    T_lo = int(ntiles[:, 0].sum())
    T_hi = int(ntiles[:, 1].sum())
    T = T_lo + T_hi
    lo_start = np.concatenate([[0], np.cumsum(ntiles[:, 0])])
    hi_start = np.concatenate([[0], np.cumsum(ntiles[:, 1])]) + T_lo

    idx_lo = np.zeros((NCORES, T_lo * TILE), np.int64)
    idx_hi = np.zeros((NCORES, T_hi * TILE), np.int64)
    dstw = np.full((NCORES, T * TILE), -1.0, np.float32)
    for d in range(NCORES):
        s_d, t_d, ofs = per_core[d]
        for w in range(W):
            for ph in range(2):
                a, b = ofs[ph * W + w], ofs[ph * W + w + 1]
                n = b - a
                stream_t = lo_start[w] if ph == 0 else hi_start[w]
                sb = stream_t * TILE
                blk = s_d[a:b] // NPB
                off = s_d[a:b] % NPB
                hidx = blk * (NPB // 2) + off % (NPB // 2)
                if ph == 0:
                    idx_lo[d, sb:sb + n] = hidx
                else:
                    idx_hi[d, (sb - T_lo * TILE):(sb - T_lo * TILE) + n] = hidx
                dstw[d, sb:sb + n] = (t_d[a:b] & 127).astype(np.float32)

    def wrap16(arr):  # [n] -> [128, n//16]: 16-partition wrap replicated x8
        w = arr.reshape(-1, 16).T.astype(np.int16)
        return np.ascontiguousarray(np.tile(w, (8, 1)))

    return {
        "ntiles": ntiles, "T_lo": T_lo, "T_hi": T_hi, "T": T,
        "lo_start": lo_start, "hi_start": hi_start, "perm": perm,
        "idx_lo_w": [wrap16(idx_lo[d]) for d in range(NCORES)],
        "idx_hi_w": [wrap16(idx_hi[d]) for d in range(NCORES)],
        "dstw": dstw,
    }


def plan_calls(n_tiles):
    calls, t = [], 0
    while t < n_tiles:
        n = min(GCALL, n_tiles - t)
        calls.append((t, n))
        t += n
    return calls


# ----------------------------------------------------------------------------
# device program
# ----------------------------------------------------------------------------

def build_nc(meta, debug=False, nlayers=2):
    ntiles = meta["ntiles"]
    T_lo, T_hi, T = meta["T_lo"], meta["T_hi"], meta["T"]
    lo_start, hi_start = meta["lo_start"], meta["hi_start"]

    nc = bacc.Bacc("TRN2", target_bir_lowering=False, debug=debug,
                   num_swdge_queues=2)

    # ---- I/O ----
    xT = nc.dram_tensor("xT", [IDIM, NPB], BF16, kind="ExternalInput")
    ew = {}
    for item in [
        ("enc_w0", [IDIM, HLD], BF16), ("enc_b0", [HLD, 1]),
        ("enc_w1", [HLD, HLD], BF16), ("enc_b1", [HLD, 1]),
        ("dec_w0", [HLD, HLD], BF16), ("dec_b0", [HLD, 1]),
        ("dec_w1", [HLD, ODIM], BF16), ("dec_b1", [ODIM, 1]),
        ("wl0", [HLD + 1, FDIM], BF16), ("wr0", [HLD + 1, FDIM], BF16),
        ("wl1", [HLD + 1, FDIM], BF16), ("wr1", [HLD + 1, FDIM], BF16),
        ("att0", [128, 4 * FDIM], BF16), ("att1", [128, 4 * FDIM], BF16),
        ("gbias0", [128, HLD]), ("gbias1", [128, HLD]),
        ("identb", [128, 128], BF16),
    ]:
        nm, shape = item[0], item[1]
        dt = item[2] if len(item) > 2 else F32
        ew[nm] = nc.dram_tensor(nm, shape, dt, kind="ExternalInput")
    idx_lo_d = nc.dram_tensor("idx_lo", [128, T_lo * 8], I16, kind="ExternalInput")
    idx_hi_d = nc.dram_tensor("idx_hi", [128, T_hi * 8], I16, kind="ExternalInput")
    smt_d = nc.dram_tensor("smt", [128, T * TILE], BF16, kind="ExternalInput")
    smat_d = nc.dram_tensor("smat", [128, T * TILE], BF16, kind="ExternalInput")
    out_d = nc.dram_tensor("outT", [ODIM, NPB], F32, kind="ExternalOutput")

    # internal DRAM (half-tables H1/H2, double-buffered across layers)
    xl_local = nc.dram_tensor("xl_local", [NPB, XLP], BF16)
    tabs = [[nc.dram_tensor("tab%d_%d" % (h, p), [LO, XLP], BF16,
                            addr_space="Shared")
             for h in range(2)] for p in range(2)]

    lo_calls = plan_calls(T_lo)
    hi_calls = plan_calls(T_hi)

    with tile.TileContext(nc) as tc:
        import contextlib
        stack = contextlib.ExitStack()
        pool = lambda *a, **k: stack.enter_context(tc.tile_pool(*a, **k))
        constp = pool(name="const", bufs=1)
        tabp = pool(name="tables", bufs=1)
        bigp = pool(name="bigbuf", bufs=1)
        xtp = pool(name="xt", bufs=3)
        htp = pool(name="ht", bufs=1)
        hnp = pool(name="hnext", bufs=1)
        stagep = pool(name="stage", bufs=4)
        glop = pool(name="glo", bufs=4)
        ghip = pool(name="ghi", bufs=4)
        smtp = pool(name="smt", bufs=3)
        smatp = pool(name="smat", bufs=3)
        edgep = pool(name="edge", bufs=4)
        decp = pool(name="dec", bufs=2)
        finp = pool(name="fin", bufs=2)
        psp = pool(name="ps", bufs=2, space="PSUM")
        psaccp = pool(name="psacc", bufs=2, space="PSUM")
        pssqp = pool(name="pssq", bufs=3, space="PSUM")
        if True:
            # ---- persistent SBUF ----
            consts = {}
            for nm in ["enc_w0", "enc_b0", "enc_w1", "enc_b1", "dec_w0",
                       "dec_b0", "dec_w1", "dec_b1", "wl0", "wr0", "wl1",
                       "wr1", "att0", "att1", "gbias0", "gbias1", "identb"]:
                t = constp.tile(list(ew[nm].shape), ew[nm].dtype, tag=nm)
                nc.sync.dma_start(t[:], ew[nm][:])
                consts[nm] = t

            ilo_sb = tabp.tile([128, T_lo * 8], I16, tag="ilo")
            nc.sync.dma_start(ilo_sb[:], idx_lo_d[:])
            ihi_sb = tabp.tile([128, T_hi * 8], I16, tag="ihi")
            nc.sync.dma_start(ihi_sb[:], idx_hi_d[:])
            xr_sbs = [tabp.tile([128, W * FDIM], BF16, tag="xr_sb%d" % i,
                                name="xr_sb%d" % i)
                      for i in range(2)]

            # ---- encoder ----
            h0T = bigp.tile([HLD, NPB], BF16, tag="big")
            for t in range(W):
                xt_t = xtp.tile([IDIM, 128], BF16, tag="xt")
                nc.sync.dma_start(xt_t[:], xT[:, t * 128:(t + 1) * 128])
                ps = psp.tile([HLD, 128], F32, tag="ps")
                nc.tensor.matmul(ps[:], lhsT=consts["enc_w0"][:],
                                 rhs=xt_t[:], start=True, stop=True)
                nc.scalar.activation(h0T[:, t * 128:(t + 1) * 128], ps[:],
                                     mybir.ActivationFunctionType.Relu,
                                     bias=consts["enc_b0"][:, 0:1])
            hT = htp.tile([HLD + 1, NPB], BF16, tag="hT")
            nc.vector.memset(hT[HLD:HLD + 1, :], 1.0)
            for t in range(W):
                ps = psp.tile([HLD, 128], F32, tag="ps")
                nc.tensor.matmul(ps[:], lhsT=consts["enc_w1"][:],
                                 rhs=h0T[:, t * 128:(t + 1) * 128],
                                 start=True, stop=True)
                nc.scalar.activation(hT[0:HLD, t * 128:(t + 1) * 128], ps[:],
                                     mybir.ActivationFunctionType.Relu,
                                     bias=consts["enc_b1"][:, 0:1])

            HALF = NPB // 2

            def emit_ag(half, layer):
                nc.gpsimd.collective_compute(
                    "AllGather", mybir.AluOpType.bypass,
                    replica_groups=[list(range(NCORES))],
                    ins=[xl_local[half * HALF:(half + 1) * HALF, :]],
                    outs=[tabs[layer % 2][half][:]],
                )

            def build_lw(hT_cur, t, layer):
                """xl/xr dense build for window t of `layer` from hT_cur.
                Emits the half-table AllGathers as their inputs complete."""
                wl = consts["wl%d" % layer]
                wr = consts["wr%d" % layer]
                lhsT = hT_cur[:, t * 128:(t + 1) * 128]
                psl = psp.tile([128, FDIM], F32, tag="ps")
                nc.tensor.matmul(psl[:], lhsT=lhsT, rhs=wl[:],
                                 start=True, stop=True)
                st = stagep.tile([128, FDIM], BF16, tag="bldst")
                nc.scalar.activation(st[:], psl[:],
                                     mybir.ActivationFunctionType.Copy)
                nc.sync.dma_start(xl_local[t * 128:(t + 1) * 128, 0:FDIM],
                                  st[:])
                psr = psp.tile([128, FDIM], F32, tag="ps")
                nc.tensor.matmul(psr[:], lhsT=lhsT, rhs=wr[:],
                                 start=True, stop=True)
                nc.scalar.activation(
                    xr_sbs[layer % 2][:, t * FDIM:(t + 1) * FDIM], psr[:],
                    mybir.ActivationFunctionType.Copy)
                if t == (W // 2):
                    emit_ag(0, layer)
                elif t == W - 1:
                    emit_ag(1, layer)

            def decode_win(hT_cur, t):
                """decoder MLP for window t, folded into the edge phase."""
                ps0 = psp.tile([HLD, 128], F32, tag="ps")
                nc.tensor.matmul(ps0[:], lhsT=consts["dec_w0"][:],
                                 rhs=hT_cur[0:HLD, t * 128:(t + 1) * 128],
                                 start=True, stop=True)
                y0 = decp.tile([HLD, 128], BF16, tag="y0")
                nc.scalar.activation(y0[:], ps0[:],
                                     mybir.ActivationFunctionType.Relu,
                                     bias=consts["dec_b0"][:, 0:1])
                ps1 = psp.tile([ODIM, 128], F32, tag="ps")
                nc.tensor.matmul(ps1[:], lhsT=consts["dec_w1"][:],
                                 rhs=y0[:], start=True, stop=True)
                yo = decp.tile([ODIM, 128], F32, tag="yo")
                nc.scalar.activation(yo[:], ps1[:],
                                     mybir.ActivationFunctionType.Relu,
                                     bias=consts["dec_b1"][:, 0:1])
                nc.sync.dma_start(out_d[:, t * 128:(t + 1) * 128], yo[:])

            # upfront build of layer 0 (layer 1 builds fold into edge 0)
            for t in range(W):
                build_lw(hT, t, 0)

            # ---- GAT layers ----
            for layer in range(nlayers):
                att = consts["att%d" % layer]
                gbias = consts["gbias%d" % layer]
                xr_sb = xr_sbs[layer % 2]
                tabH1 = tabs[layer % 2][0]
                tabH2 = tabs[layer % 2][1]

                # gather + onehot-mat prefetch (emitted lazily)
                lo_bufs = [None] * len(lo_calls)
                hi_bufs = [None] * len(hi_calls)

                def emit(ci, calls, bufs, idx_sb, tab, pool_, qn, gtag):
                    t0, n = calls[ci]
                    b = pool_.tile([128, GCALL, XLP], BF16, tag=gtag)
                    nc.gpsimd.dma_gather(
                        b[:, 0:n, :], tab, idx_sb[:, t0 * 8:(t0 + n) * 8],
                        n * TILE, n * TILE, XLP, single_packet=False,
                        queue_num=qn)
                    smt = smtp.tile([128, GCALL * TILE], BF16, tag="smt")
                    sm = smatp.tile([128, GCALL * TILE], BF16, tag="smat")
                    if gtag == "ghi":
                        t0 = t0 + T_lo
                    nc.sync.dma_start(
                        smt[:, 0:n * TILE],
                        smt_d[:, t0 * TILE:(t0 + n) * TILE])
                    nc.sync.dma_start(
                        sm[:, 0:n * TILE],
                        smat_d[:, t0 * TILE:(t0 + n) * TILE])
                    bufs[ci] = (b, smt, sm)

                next_lo = [0]
                next_hi = [0]

                def get_chunk(stream_t, nb):
                    """views (xl[128,nb,192], smt[128,nb*128], sm[128,nb*128])
                    for nb consecutive stream tiles starting at stream_t."""
                    if stream_t < T_lo:
                        ci, off = divmod(stream_t, GCALL)
                        while next_lo[0] <= ci:
                            emit(next_lo[0], lo_calls, lo_bufs, ilo_sb,
                                 tabH1[:], glop, 0, "glo")
                            next_lo[0] += 1
                        b, smt, sm = lo_bufs[ci]
                    else:
                        st = stream_t - T_lo
                        ci, off = divmod(st, GCALL)
                        while next_hi[0] <= ci:
                            emit(next_hi[0], hi_calls, hi_bufs, ihi_sb,
                                 tabH2[:], ghip, 1, "ghi")
                            next_hi[0] += 1
                        b, smt, sm = hi_bufs[ci]
                    return (b[:, off:off + nb, 0:FDIM],
                            smt[:, off * TILE:(off + nb) * TILE],
                            sm[:, off * TILE:(off + nb) * TILE])

                h_next = hnp.tile([128, W * HLD], BF16, tag="hnext")
                hTn = htp.tile([HLD + 1, NPB], BF16, tag="hT")
                nc.vector.memset(hTn[HLD:HLD + 1, :], 1.0)

                def window_chunks(w, ph):
                    chunks = []
                    base = lo_start[w] if ph == 0 else hi_start[w]
                    cnt = ntiles[w, ph]
                    t = base
                    while t < base + cnt:
                        pos = t if t < T_lo else t - T_lo
                        nb = min(base + cnt - t, 4, GCALL - pos % GCALL)
                        chunks.append((int(t), int(nb)))
                        t += nb
                    return chunks

                def edge_chunk(b0, nb, xr_win, acc, wi, first, last):
                    xl_q, smt_q, sm_q = get_chunk(b0, nb)
                    wm_q = edgep.tile([128, 4, 200], BF16, tag="wm")
                    m0_q = edgep.tile([128, 4, FDIM], BF16, tag="m0")
                    # s = xl + xr[dst] in PSUM (identity pass + onehot matmul)
                    for sb0 in range(0, nb, 2):
                        ns = min(2, nb - sb0)
                        psq = pssqp.tile([128, 2, FDIM], F32, tag="sqd")
                        nc.tensor.matmul(
                            psq[:, 0:ns, :], lhsT=consts["identb"][:],
                            rhs=xl_q[:, sb0:sb0 + ns, :],
                            start=True, stop=False)
                        for t in range(ns):
                            tt = sb0 + t
                            nc.tensor.matmul(
                                psq[:, t, :],
                                lhsT=smt_q[:, tt * TILE:(tt + 1) * TILE],
                                rhs=xr_win, start=False, stop=True)
                        nc.scalar.activation(
                            m0_q[:, sb0:sb0 + ns, :], psq[:, 0:ns, :],
                            mybir.ActivationFunctionType.Prelu, alpha=SLOPE)
                    m_q = edgep.tile([128, 4, FDIM], BF16, tag="m")
                    nc.vector.tensor_tensor(
                        out=m_q[:, 0:nb, :],
                        in0=m0_q[:, 0:nb, :],
                        in1=att[:].rearrange("p (t f) -> p t f", t=4)
                            [:, 0:nb, :],
                        op=mybir.AluOpType.mult)
                    pe_q = edgep.tile([128, 4, 8], F32, tag="pe")
                    nc.vector.tensor_reduce(
                        out=pe_q[:, 0:nb, 4:7],
                        in_=m_q[:, 0:nb, :].rearrange(
                            "p t (h c) -> p t h c", h=H),
                        axis=mybir.AxisListType.X,
                        op=mybir.AluOpType.add)
                    nc.scalar.activation(
                        wm_q[:, 0:nb, FDIM:FDIM + H], pe_q[:, 0:nb, 4:7],
                        mybir.ActivationFunctionType.Exp)
                    nc.vector.tensor_tensor(
                        out=wm_q[:, 0:nb, 0:FDIM].rearrange(
                            "p t (h c) -> p t h c", h=H),
                        in0=xl_q.rearrange("p t (h c) -> p t h c", h=H),
                        in1=wm_q[:, 0:nb, FDIM:FDIM + H].broadcast_to(
                            [128, nb, H, HLD]),
                        op=mybir.AluOpType.mult)
                    for t in range(nb):
                        nc.tensor.matmul(
                            acc[:, wi, 0:ACC_COLS],
                            lhsT=sm_q[:, t * TILE:(t + 1) * TILE],
                            rhs=wm_q[:, t, 0:ACC_COLS],
                            start=(first and t == 0),
                            stop=(last and t == nb - 1))

                for wp in range(0, W, 2):
                    wpair = [w for w in (wp, wp + 1) if w < W]
                    acc = psaccp.tile([128, 2, 256], F32, tag="acc")
                    for wi, w in enumerate(wpair):
                        xr_win = xr_sb[:, w * FDIM:(w + 1) * FDIM]
                        chunks = (window_chunks(w, 0)
                                  + window_chunks(w, 1))
                        nch = len(chunks)
                        for ci_, (b0, nb) in enumerate(chunks):
                            edge_chunk(b0, nb, xr_win, acc, wi,
                                       ci_ == 0, ci_ == nch - 1)

                    # ---- finalize window pair ----
                    np_ = len(wpair)
                    fin = finp.tile([128, 2, 8], F32, tag="fin")
                    nc.vector.tensor_scalar(
                        out=fin[:, 0:np_, 0:3], in0=acc[:, 0:np_, 192:195],
                        scalar1=3.0, scalar2=1e-16,
                        op0=mybir.AluOpType.mult, op1=mybir.AluOpType.add)
                    nc.vector.reciprocal(fin[:, 0:np_, 4:7],
                                         fin[:, 0:np_, 0:3])
                    u_t = finp.tile([128, 2, FDIM], F32, tag="u")
                    for h in range(H):
                        nc.vector.tensor_tensor(
                            out=u_t[:, 0:np_, h * HLD:(h + 1) * HLD],
                            in0=acc[:, 0:np_, h * HLD:(h + 1) * HLD],
                            in1=fin[:, 0:np_, 4 + h:5 + h].broadcast_to(
                                [128, np_, HLD]),
                            op=mybir.AluOpType.mult)
                    v_t = finp.tile([128, 2, HLD], F32, tag="v")
                    nc.vector.tensor_tensor(
                        out=v_t[:, 0:np_, :], in0=u_t[:, 0:np_, 0:HLD],
                        in1=u_t[:, 0:np_, HLD:2 * HLD],
                        op=mybir.AluOpType.add)
                    v2_t = finp.tile([128, 2, HLD], F32, tag="v2")
                    nc.vector.tensor_tensor(
                        out=v2_t[:, 0:np_, :], in0=v_t[:, 0:np_, :],
                        in1=u_t[:, 0:np_, 2 * HLD:3 * HLD],
                        op=mybir.AluOpType.add)
                    v3_t = finp.tile([128, 2, HLD], F32, tag="v3")
                    nc.vector.tensor_tensor(
                        out=v3_t[:, 0:np_, :], in0=v2_t[:, 0:np_, :],
                        in1=gbias[:].rearrange("p (o f) -> p o f", o=1)
                            .broadcast_to([128, np_, HLD]),
                        op=mybir.AluOpType.add)
                    nc.scalar.activation(
                        h_next[:, wp * HLD:(wp + np_) * HLD],
                        v3_t[:, 0:np_, :],
                        mybir.ActivationFunctionType.Relu)

                    # ---- fold transpose + next dense phase per window ----
                    for w in wpair:
                        pst = psp.tile([HLD, 128], BF16, tag="pst", bufs=1)
                        nc.tensor.transpose(
                            pst[:], in_=h_next[:, w * HLD:(w + 1) * HLD],
                            identity=consts["identb"][:])
                        nc.scalar.activation(
                            hTn[0:HLD, w * 128:(w + 1) * 128], pst[:],
                            mybir.ActivationFunctionType.Copy)
                        if layer + 1 < nlayers:
                            build_lw(hTn, w, layer + 1)
                        else:
                            decode_win(hTn, w)

                hT = hTn

        stack.close()

    nc.compile()
    return nc


# ----------------------------------------------------------------------------
# host orchestration
# ----------------------------------------------------------------------------

def make_in_maps(inputs, plan):
    x = np.asarray(inputs["x"], np.float32)
    xpad = np.zeros((NPAD, IDIM), np.float32)
    xpad[:N] = x
    xpad = xpad[plan["perm"]]

    def col(b):
        return np.ascontiguousarray(np.asarray(b, np.float32).reshape(-1, 1))

    def wplus(wname, bname):
        wm = np.asarray(inputs[wname], np.float32)
        bm = np.asarray(inputs[bname], np.float32)
        return _bf16(np.vstack([wm, bm[None, :]]))

    shared = {
        "enc_w0": _bf16(inputs["enc_w0"]), "enc_b0": col(inputs["enc_b0"]),
        "enc_w1": _bf16(inputs["enc_w1"]), "enc_b1": col(inputs["enc_b1"]),
        "dec_w0": _bf16(inputs["dec_w0"]), "dec_b0": col(inputs["dec_b0"]),
        "dec_w1": _bf16(inputs["dec_w1"]), "dec_b1": col(inputs["dec_b1"]),
        "wl0": wplus("gat0_wl", "gat0_bl"),
        "wr0": wplus("gat0_wr", "gat0_br"),
        "wl1": wplus("gat1_wl", "gat1_bl"),
        "wr1": wplus("gat1_wr", "gat1_br"),
        "att0": _bf16(np.tile(np.asarray(inputs["gat0_att"], np.float32)
                              .reshape(1, FDIM), (128, 4))),
        "att1": _bf16(np.tile(np.asarray(inputs["gat1_att"], np.float32)
                              .reshape(1, FDIM), (128, 4))),
        "gbias0": np.ascontiguousarray(
            np.tile(np.asarray(inputs["gat0_bias"], np.float32)
                    .reshape(1, HLD), (128, 1))),
        "gbias1": np.ascontiguousarray(
            np.tile(np.asarray(inputs["gat1_bias"], np.float32)
                    .reshape(1, HLD), (128, 1))),
        "identb": _bf16(np.eye(128, dtype=np.float32)),
    }
    maps = []
    for d in range(NCORES):
        m = dict(shared)
        m["xT"] = _bf16(xpad[d * NPB:(d + 1) * NPB].T)
        m["idx_lo"] = plan["idx_lo_w"][d]
        m["idx_hi"] = plan["idx_hi_w"][d]
        dw = plan["dstw"][d]  # [T*TILE] float (dst-in-window, -1 pad)
        # smat[p=edge-in-tile, t*128+slot] = 1{dst(edge p of tile t)==slot}
        sm = (dw.reshape(-1, TILE).T[:, :, None]
              == np.arange(128, dtype=np.float32)[None, None])
        m["smat"] = _bf16(sm.reshape(TILE, -1))
        # smt[p=slot, t*128+e] = 1{dst(edge e of tile t)==slot}
        smt = (np.arange(128, dtype=np.float32)[:, None] == dw[None, :])
        m["smt"] = _bf16(smt)
        maps.append(m)
    return maps


def kernel(**inputs):
    global LAST_EXEC_NS
    trace = inputs.pop("trace", False)
    tmpdir = inputs.pop("tmpdir", None)
    from concourse.bass_utils import run_bass_kernel_spmd

    plan = build_edge_plan(np.asarray(inputs["edgeIdx"]))
    nc = build_nc(plan)
    in_maps = make_in_maps(inputs, plan)
    res = run_bass_kernel_spmd(nc, in_maps, list(range(NCORES)),
                               tmpdir=tmpdir, trace=trace)
    LAST_EXEC_NS = res.exec_time_ns
    outs = res.results
    full = np.concatenate([outs[d]["outT"].T for d in range(NCORES)], 0)
    out = np.empty((NPAD, ODIM), np.float32)
    out[plan["perm"]] = full
    return np.ascontiguousarray(out[:N]).astype(np.float32)
